# revision 12
# baseline (speedup 1.0000x reference)
"""Bidirectional attention (RoPE-variant) Trainium2 kernel.

Reference computation (B=4, T=2048, C=2048, H=16, D=128):
    q = (x @ wq.T) -> rotary; k = (x @ wk.T) -> rotary; v = x @ wv.T
    y = softmax(q k^T / sqrt(D)) v ; out = y @ wo.T

Sharding over 8 NeuronCores: core c -> (batch b = c//2, head-group g = c%2).
Each core computes q/k/v projections for its batch restricted to its 8 heads,
full attention for those heads, and a partial o-projection (contracting its
1024 hidden columns).  The host sums the two partial outputs per batch — no
device collectives, and every core does exactly 1/8 of the matmul FLOPs.

Schedule: V is produced first (two wv quarters), then per head-window h the
Q/K projection chains for head h are emitted with attention micro-blocks for
head h-1 interleaved between them: SC(qc) = scores+exp for one 512-query
chunk, AV(qc) = tree-sum + attn@V + denominator-reduce + normalize.  The PE
queue is in-order, so this interleave is what lets the ScalarE exp() time
(~38us/window) hide under projection matmuls; AV(qc) is placed ~3 chains
after SC(qc) so exp has drained by then.  K^T stays resident in SBUF in f32
(rotary writes it directly; no spill), Q^T spills to DRAM in f32, and the
scores matmul consumes both as float32r — same PE throughput as bf16 at
N=512 (~227ns vs 216ns measured) with ~18x better precision, eliminating
the q/k quantization error that dominates exp(scores).  The softmax
denominator's cross-partition reduce is a single PE matmul against an
all-ones stationary operand.  The partial o-projection interleaves into the
last head's attention and is written out in bf16 (host accumulates in f32).
"""

import sys

if "/opt/trn_rl_repo" not in sys.path:
    sys.path.insert(0, "/opt/trn_rl_repo")

import numpy as np
import ml_dtypes

B, T, C = 4, 2048, 2048
H_TOT = 16
D = 128
HG = 8            # heads per core
JG = HG * D       # 1024 hidden columns per head-group
N_CORES = 8
CT = C // 128     # 16 c-tiles (contraction over channels)
TT = T // 128     # 16 t-tiles
QCH = T // 512    # 4 query chunks of 512
KT = T // 128     # 16 key tiles of 128
VQ = JG // 256    # 4 wv quarters
SCALE = 1.0 / float(np.sqrt(D))

BF16 = ml_dtypes.bfloat16

_CACHE = {}


def _build_bass():
    import concourse.tile as tile
    from concourse import bacc, mybir
    from concourse.bass import ts
    from contextlib import ExitStack

    bf16 = mybir.dt.bfloat16
    f32 = mybir.dt.float32
    f32r = mybir.dt.float32r

    nc = bacc.Bacc("TRN2", target_bir_lowering=False, debug=False)

    # x/wv are packed chunk-major so each load is one DMA with fat
    # per-partition-contiguous descriptors on both sides — startup is
    # DMA-bound, so descriptor efficiency sets the PE start time.
    x_pack = nc.dram_tensor("x_pack", [QCH, 128, CT, 512], bf16, kind="ExternalInput")
    wq_pack = nc.dram_tensor("wq_pack", [HG, 128, CT, 128], bf16, kind="ExternalInput")
    wk_pack = nc.dram_tensor("wk_pack", [HG, 128, CT, 128], bf16, kind="ExternalInput")
    wv_pack = nc.dram_tensor("wv_pack", [VQ, 128, CT, 256], bf16, kind="ExternalInput")
    wo_pack = nc.dram_tensor("wo_pack", [128, HG, C], bf16, kind="ExternalInput")
    # cs_pack rows 0:64 = cos^T, rows 64:128 = sin^T
    cs_pack = nc.dram_tensor("cs_pack", [128, T], f32, kind="ExternalInput")
    out = nc.dram_tensor("out", [T, C], bf16, kind="ExternalOutput")

    with tile.TileContext(nc) as tc, ExitStack() as ctx:
        # Pools opened in lifetime order: persistent + attention scratch first
        # (bottom of the SBUF stack), then phase-1 pools on top, so attention
        # tiles never alias phase-1 space.
        persist = ctx.enter_context(tc.tile_pool(name="persist", bufs=1))
        p2k = ctx.enter_context(tc.tile_pool(name="p2k", bufs=2))
        p2q = ctx.enter_context(tc.tile_pool(name="p2q", bufs=2))
        p2u = ctx.enter_context(tc.tile_pool(name="p2u", bufs=2))
        p2sm = ctx.enter_context(tc.tile_pool(name="p2sm", bufs=1))
        p2v = ctx.enter_context(tc.tile_pool(name="p2v", bufs=1))
        dram = ctx.enter_context(tc.tile_pool(name="dram", bufs=1, space="DRAM"))
        ps_sc = ctx.enter_context(tc.tile_pool(name="ps_sc", bufs=2, space="PSUM"))
        ps_gen = ctx.enter_context(tc.tile_pool(name="ps_gen", bufs=3, space="PSUM"))
        ps_red = ctx.enter_context(tc.tile_pool(name="ps_red", bufs=1, space="PSUM"))

        yt_sb = persist.tile([128, HG, T], bf16)      # y^T, (d, h, t)
        ones_sb = persist.tile([128, 128], bf16, tag="ones")
        nc.vector.memset(ones_sb[:], 1.0)

        qt_dram = [
            dram.tile([128, T], f32r, tag=f"qt{h}", name=f"qt_dram{h}")
            for h in range(HG)
        ]
        v_dram = dram.tile([128, TT, JG], bf16, name="v_dram")

        # ---- attention micro-block emitters (head h, interleaved) --------
        class Att:
            def __init__(self, h, kt_sb):
                self.h = h
                self.kt = kt_sb
                v_h = p2v.tile([128, TT, 128], bf16, tag="vh", name=f"vh{h}")
                nc.sync.dma_start(out=v_h[:], in_=v_dram[:, :, ts(h, 128)])
                self.v = v_h
                self.qts = {}
                self.us = {}
                self.prefetch_qt(0)

            def prefetch_qt(self, qc):
                if qc >= QCH or qc in self.qts:
                    return
                qt = p2q.tile(
                    [128, 512], f32r, tag="qt", name=f"qt{self.h}_{qc}"
                )
                nc.sync.dma_start(
                    out=qt[:], in_=qt_dram[self.h][:, ts(qc, 512)]
                )
                self.qts[qc] = qt

            def sc(self, qc):
                h = self.h
                self.prefetch_qt(qc + 1)
                u = p2u.tile(
                    [128, KT // 2, 2, 512], bf16, tag="u", name=f"u{h}_{qc}"
                )
                self.us[qc] = u
                qt_r = self.qts[qc][:]
                for kg in range(KT // 2):
                    ps = ps_sc.tile(
                        [128, 2, 512], f32, tag="ps", name=f"sc{h}_{qc}_{kg}"
                    )
                    for kk in range(2):
                        nc.tensor.matmul(
                            ps[:, kk, :],
                            lhsT=self.kt[:, ts(2 * kg + kk, 128)],
                            rhs=qt_r,
                            start=True,
                            stop=True,
                        )
                    nc.scalar.activation(
                        out=u[:, kg, :, :],
                        in_=ps[:],
                        func=mybir.ActivationFunctionType.Exp,
                        scale=SCALE,
                    )

            def av(self, qc):
                h = self.h
                u = self.us.pop(qc)
                self.qts.pop(qc)
                # denominator tree-sum on VectorE; exp(qc) has drained by the
                # time this block is emitted, so these don't head-block the
                # (in-order) Vector queue
                s8 = p2sm.tile([128, 8, 512], bf16, tag="s8", name=f"s8_{h}{qc}")
                nc.vector.tensor_add(s8[:], u[:, :, 0, :], u[:, :, 1, :])
                s8v = s8[:].rearrange("p (x y) q -> p x y q", x=4)
                s4 = p2sm.tile([128, 4, 512], bf16, tag="s4", name=f"s4_{h}{qc}")
                nc.vector.tensor_add(s4[:], s8v[:, :, 0, :], s8v[:, :, 1, :])
                s4v = s4[:].rearrange("p (x y) q -> p x y q", x=2)
                s2r = p2sm.tile([128, 3, 512], bf16, tag="s2r", name=f"s2r_{h}{qc}")
                nc.vector.tensor_add(
                    s2r[:, 0:2, :], s4v[:, :, 0, :], s4v[:, :, 1, :]
                )
                nc.vector.tensor_add(s2r[:, 2, :], s2r[:, 0, :], s2r[:, 1, :])

                psy = ps_gen.tile([128, 512], f32, tag="ps", name=f"psy{h}_{qc}")
                for kt in range(KT):
                    nc.tensor.matmul(
                        psy[:],
                        lhsT=self.v[:, kt, :],
                        rhs=u[:, kt // 2, kt % 2, :],
                        start=(kt == 0),
                        stop=(kt == KT - 1),
                    )
                # cross-partition reduce of the denominator: one all-ones
                # matmul (~0.2us) instead of a ~3.5us gpsimd all-reduce
                rsum = ps_red.tile([128, 512], f32, tag="red", name=f"rs_{h}{qc}")
                nc.tensor.matmul(
                    rsum[:], lhsT=ones_sb[:], rhs=s2r[:, 2, :],
                    start=True, stop=True,
                )
                rrec = p2sm.tile([128, 512], f32, tag="s4", name=f"rr_{h}{qc}")
                nc.vector.reciprocal_approx_fast(out=rrec[:], in_=rsum[:])
                nc.vector.tensor_mul(
                    out=yt_sb[:, h, ts(qc, 512)], in0=psy[:], in1=rrec[:]
                )

        def oproj_group(g):
            # o-proj tile-group g needs q-chunk g of ALL heads; emitted in
            # the tail right after yt(7, g) is finalized.  Two 512-col
            # halves at a time so only 2 PSUM banks are held; each half's
            # output is DMA'd as soon as its copies finish.
            for tm in range(4 * g, 4 * g + 4):
                stg = p3stg.tile([128, C], bf16, tag="ostg", name=f"ostg{tm}")
                for half in range(2):
                    pss = [
                        ps_gen.tile(
                            [128, 512], f32, tag="ps", name=f"pso{tm}_{half}{c}"
                        )
                        for c in range(2)
                    ]
                    for ji in range(HG):
                        for c2 in range(2):
                            nc.tensor.matmul(
                                pss[c2][:],
                                lhsT=yt_sb[:, ji, ts(tm, 128)],
                                rhs=wo_sb[:, ji, ts(2 * half + c2, 512)],
                                start=(ji == 0),
                                stop=(ji == HG - 1),
                            )
                    for c2 in range(2):
                        cch = 2 * half + c2
                        nc.vector.tensor_copy(
                            out=stg[:, ts(cch, 512)], in_=pss[c2][:]
                        )
                        nc.sync.dma_start(
                            out=out.ap()[ts(tm, 128), ts(cch, 512)],
                            in_=stg[:, ts(cch, 512)],
                        )

        # ---- phase 1: projections with interleaved attention -------------
        with (
            tc.tile_pool(name="p1x", bufs=1) as p1x,
            tc.tile_pool(name="p1wv", bufs=2) as p1wv,
            tc.tile_pool(name="p1cs", bufs=1) as p1cs,
            tc.tile_pool(name="p1w", bufs=1) as p1w,
            tc.tile_pool(name="p1rot", bufs=1) as p1rot,
            tc.tile_pool(name="p1stg", bufs=2) as p1stg,
        ):
            def load_w(h):
                w_h = {}
                for nm, pack in (("q", wq_pack), ("k", wk_pack)):
                    w = p1w.tile(
                        [128, CT, 128], bf16, tag=f"w{nm}", name=f"w{nm}{h}"
                    )
                    nc.sync.dma_start(out=w[:], in_=pack.ap()[h])
                    w_h[nm] = w
                return w_h

            def load_wv(qd):
                wv_h = p1wv.tile(
                    [128, CT, 256], bf16, tag="wvh", name=f"wvh{qd}"
                )
                nc.sync.dma_start(out=wv_h[:], in_=wv_pack.ap()[qd])
                return wv_h

            # DMA issue order = DMA queue order (single in-order HW queue
            # striped over all 16 engines).  The first V chain needs only
            # wv quarter 0 + x chunk 0 (~2.6MB); later x chunks are emitted
            # lazily from inside v_quarter(0) so the V spill stores
            # interleave into the queue instead of stalling behind 8MB of
            # loads (vstg recycling would otherwise block the V chains).
            wv_h0 = load_wv(0)
            x_sb = p1x.tile([128, QCH, CT, 512], bf16, tag="xt")
            nc.sync.dma_start(out=x_sb[:, 0, :, :], in_=x_pack.ap()[0])
            w_next = load_w(0)
            nc.sync.dma_start(out=x_sb[:, 1, :, :], in_=x_pack.ap()[1])
            wv_h1 = load_wv(1)

            def v_quarter(qd, wv_h, xload=None):
                # V columns [qd*256, qd*256+256) for all T (heads 2qd, 2qd+1)
                for tm in range(TT):
                    ps = ps_gen.tile(
                        [128, 256], f32, tag="ps", name=f"vps{qd}_{tm}"
                    )
                    for ci in range(CT):
                        nc.tensor.matmul(
                            ps[:],
                            lhsT=x_sb[:, tm // 4, ci, ts(tm % 4, 128)],
                            rhs=wv_h[:, ci, :],
                            start=(ci == 0),
                            stop=(ci == CT - 1),
                        )
                    vstg = p1stg.tile(
                        [128, 256], bf16, tag="vst", bufs=2,
                        name=f"vstg{qd}_{tm}"
                    )
                    nc.scalar.copy(out=vstg[:], in_=ps[:])
                    nc.sync.dma_start(
                        out=v_dram[:, tm, ts(qd, 256)], in_=vstg[:]
                    )
                    if xload and tm in xload:
                        tc4 = xload[tm]
                        nc.sync.dma_start(
                            out=x_sb[:, tc4, :, :], in_=x_pack.ap()[tc4]
                        )

            def qk_chain(h, w_h, kt_cur, nm, tch):
                ps = ps_gen.tile(
                    [128, 512], f32, tag="ps", name=f"qk{h}{nm}{tch}"
                )
                for ci in range(CT):
                    nc.tensor.matmul(
                        ps[:],
                        lhsT=w_h[nm][:, ci, :],
                        rhs=x_sb[:, tch, ci, :],
                        start=(ci == 0),
                        stop=(ci == CT - 1),
                    )
                # rotary: out1 = x1*cos + x2*sin ; out2 = x1*cos - x2*sin
                t12 = p1rot.tile([64, 2, 512], f32, tag="t12")
                t1 = t12[:, 0, :]
                t2 = t12[:, 1, :]
                nc.vector.tensor_mul(t1, ps[0:64, :], cs_sb[0:64, ts(tch, 512)])
                nc.vector.tensor_mul(
                    t2, ps[64:128, :], cs_sb[64:128, ts(tch, 512)]
                )
                if nm == "k":
                    # K^T written straight into its resident SBUF tile
                    nc.vector.tensor_add(kt_cur[0:64, ts(tch, 512)], t1, t2)
                    nc.vector.tensor_sub(kt_cur[64:128, ts(tch, 512)], t1, t2)
                else:
                    # Q^T spilled to DRAM in f32 (consumed as float32r)
                    stg = p1stg.tile(
                        [128, 512], f32r, tag="spl", bufs=1,
                        name=f"stg{h}{nm}{tch}"
                    )
                    nc.vector.tensor_add(stg[0:64, :], t1, t2)
                    nc.vector.tensor_sub(stg[64:128, :], t1, t2)
                    nc.sync.dma_start(
                        out=qt_dram[h][:, ts(tch, 512)], in_=stg[:]
                    )

            # V quarters 0,1 (heads 0-3) up front; 2,3 inside window 0
            v_quarter(0, wv_h0, xload={2: 2, 6: 3})
            # cos/sin only needed at the first rotary (~75us in)
            cs_sb = p1cs.tile([128, T], f32, tag="cs")
            nc.sync.dma_start(out=cs_sb[:], in_=cs_pack.ap())
            v_quarter(1, wv_h1)

            atts = {}
            kts = {}
            for h in range(HG):
                w_h = w_next
                kt_cur = p2k.tile([128, T], f32r, tag="kt", name=f"kt{h}")
                kts[h] = kt_cur
                if h >= 1:
                    atts[h - 1] = Att(h - 1, kts[h - 1])
                # interleave schedule: after QK chain i of window h, emit
                # attention micro-block inserts[i].  AV(qc) trails SC(qc) by
                # >=3 chains (~15us) so the ScalarE exp has drained; the
                # last two AV blocks of head h-1 ride in window h+1.
                a_prev = atts.get(h - 2)   # AV(h-2, 2/3) pending
                a_cur = atts.get(h - 1)
                inserts = [
                    (a_prev, "av", 2),
                    (a_prev, "av", 3),
                    (a_cur, "sc", 0),
                    (a_cur, "sc", 1),
                    (a_cur, "av", 0),
                    (a_cur, "sc", 2),
                    (a_cur, "av", 1),
                    (a_cur, "sc", 3),
                ]
                chains = [(nm, tch) for nm in ("q", "k") for tch in range(QCH)]
                for i, (nm, tch) in enumerate(chains):
                    qk_chain(h, w_h, kt_cur, nm, tch)
                    a, kind, qc = inserts[i]
                    if a is not None:
                        getattr(a, kind)(qc)
                    if h == 0 and nm == "q" and tch == 3:
                        v_quarter(2, load_wv(2))
                    if h == 0 and nm == "k" and tch == 3:
                        v_quarter(3, load_wv(3))
                if h + 1 < HG:
                    w_next = load_w(h + 1)
                if h - 2 in atts:
                    del atts[h - 2]

        # ---- tail: attention(7) + pending AV(6) + o-projection -----------
        p3wo = ctx.enter_context(tc.tile_pool(name="p3wo", bufs=1))
        p3stg = ctx.enter_context(tc.tile_pool(name="p3stg", bufs=2))

        a6 = atts[HG - 2]
        a7 = Att(HG - 1, kts[HG - 1])
        # only qt(0)/qt(1) fit the 2-buf ring up front; a third prefetch's
        # DMA would wait on a buffer release at the HEAD of the in-order DMA
        # queue and block the wo loads behind it
        a7.prefetch_qt(1)
        wo_sb = p3wo.tile([128, HG, C], bf16)
        for ji in range(HG):
            nc.sync.dma_start(out=wo_sb[:, ji, :], in_=wo_pack.ap()[:, ji, :])

        a6.av(2)
        a7.sc(0)
        a6.av(3)
        a7.sc(1)
        a7.av(0)
        oproj_group(0)
        a7.sc(2)
        a7.av(1)
        oproj_group(1)
        a7.sc(3)
        a7.av(2)
        oproj_group(2)
        a7.av(3)
        oproj_group(3)

    nc.compile()
    return nc


def get_nc():
    if "nc" not in _CACHE:
        _CACHE["nc"] = _build_bass()
    return _CACHE["nc"]


def _pack_inputs(x, cos, sin, wq, wk, wv, wo):
    """Build the 8 per-core input maps (packed, DMA-friendly layouts)."""
    cs = np.concatenate(
        [
            np.asarray(cos[0, :, 0, :], dtype=np.float32).T,  # (64, T)
            np.asarray(sin[0, :, 0, :], dtype=np.float32).T,
        ],
        axis=0,
    )  # (128, T)
    cs = np.ascontiguousarray(cs)
    in_maps = []
    for core in range(N_CORES):
        b, g = divmod(core, 2)
        xb = np.asarray(x[b], dtype=np.float32)  # (T, C)
        # x_pack[tc4, p, ci, t'] = x[b, tc4*512+t', ci*128+p]
        x_pack = np.ascontiguousarray(
            xb.reshape(QCH, 512, CT, 128).transpose(0, 3, 2, 1).astype(BF16)
        )
        sl = slice(g * JG, (g + 1) * JG)
        wq_g = np.asarray(wq[sl], dtype=np.float32)  # (JG, C)
        wk_g = np.asarray(wk[sl], dtype=np.float32)
        wv_g = np.asarray(wv[sl], dtype=np.float32)
        wo_g = np.asarray(wo[:, sl], dtype=np.float32)  # (C, JG)
        # wq_pack[h, ci, co, d] = wq_g[h*128+d, co*128+ci]
        wq_pack = np.ascontiguousarray(
            wq_g.reshape(HG, 128, CT, 128).transpose(0, 3, 2, 1).astype(BF16)
        )
        wk_pack = np.ascontiguousarray(
            wk_g.reshape(HG, 128, CT, 128).transpose(0, 3, 2, 1).astype(BF16)
        )
        # wv_pack[qd, ci, co, d'] = wv_g[qd*256+d', co*128+ci]
        wv_pack = np.ascontiguousarray(
            wv_g.reshape(VQ, 256, CT, 128).transpose(0, 3, 2, 1).astype(BF16)
        )
        # wo_pack[ji, jo, c] = wo_g[c, jo*128+ji]
        wo_pack = np.ascontiguousarray(
            wo_g.reshape(C, HG, 128).transpose(2, 1, 0).astype(BF16)
        )
        in_maps.append(
            {
                "x_pack": x_pack,
                "wq_pack": wq_pack,
                "wk_pack": wk_pack,
                "wv_pack": wv_pack,
                "wo_pack": wo_pack,
                "cs_pack": cs,
            }
        )
    return in_maps


def run_spmd(in_maps, **kwargs):
    from concourse.bass_utils import run_bass_kernel_spmd

    nc = get_nc()
    return run_bass_kernel_spmd(nc, in_maps, core_ids=list(range(N_CORES)), **kwargs)


def kernel(x, cos, sin, wq, wk, wv, wo):
    in_maps = _pack_inputs(x, cos, sin, wq, wk, wv, wo)
    res = run_spmd(in_maps)
    outs = [np.asarray(r["out"], dtype=np.float32) for r in res.results]
    full = np.empty((B, T, C), dtype=np.float32)
    for b in range(B):
        full[b] = outs[2 * b] + outs[2 * b + 1]
    return full


# revision 14
# speedup vs baseline: 1.1834x; 1.1834x over previous
"""Bidirectional attention (RoPE-variant) Trainium2 kernel.

Reference computation (B=4, T=2048, C=2048, H=16, D=128):
    q = (x @ wq.T) -> rotary; k = (x @ wk.T) -> rotary; v = x @ wv.T
    y = softmax(q k^T / sqrt(D)) v ; out = y @ wo.T

Sharding over 8 NeuronCores: core c -> (batch b = c//2, head-group g = c%2).
Each core computes q/k/v projections for its batch restricted to its 8 heads,
full attention for those heads, and a partial o-projection (contracting its
1024 hidden columns).  The host sums the two partial outputs per batch — no
device collectives, and every core does exactly 1/8 of the matmul FLOPs.

Schedule: V is produced first (two wv quarters), then per head-window h the
Q/K projection chains for head h are emitted with attention micro-blocks for
head h-1 interleaved between them: SC(qc) = scores+exp for one 512-query
chunk, AV(qc) = tree-sum + attn@V + denominator-reduce + normalize.  The PE
queue is in-order, so this interleave is what lets the ScalarE exp() time
(~38us/window) hide under projection matmuls; AV(qc) is placed ~3 chains
after SC(qc) so exp has drained by then.  K^T stays resident in SBUF in f32
(rotary writes it directly; no spill), Q^T spills to DRAM in f32, and the
scores matmul consumes both as float32r — same PE throughput as bf16 at
N=512 (~227ns vs 216ns measured) with ~18x better precision, eliminating
the q/k quantization error that dominates exp(scores).  The softmax
denominator's cross-partition reduce is a single PE matmul against an
all-ones stationary operand.  The partial o-projection interleaves into the
last head's attention and is written out in bf16 (host accumulates in f32).
"""

import sys

if "/opt/trn_rl_repo" not in sys.path:
    sys.path.insert(0, "/opt/trn_rl_repo")

import numpy as np
import ml_dtypes

B, T, C = 4, 2048, 2048
H_TOT = 16
D = 128
HG = 8            # heads per core
JG = HG * D       # 1024 hidden columns per head-group
N_CORES = 8
CT = C // 128     # 16 c-tiles (contraction over channels)
TT = T // 128     # 16 t-tiles
QCH = T // 512    # 4 query chunks of 512
KT = T // 128     # 16 key tiles of 128
VQ = JG // 256    # 4 wv quarters
SCALE = 1.0 / float(np.sqrt(D))

BF16 = ml_dtypes.bfloat16

_CACHE = {}


def _build_bass():
    import concourse.tile as tile
    from concourse import bacc, mybir
    from concourse.bass import ts
    from contextlib import ExitStack

    bf16 = mybir.dt.bfloat16
    f32 = mybir.dt.float32
    f32r = mybir.dt.float32r

    nc = bacc.Bacc("TRN2", target_bir_lowering=False, debug=False)

    # x/wv are packed chunk-major so each load is one DMA with fat
    # per-partition-contiguous descriptors on both sides — startup is
    # DMA-bound, so descriptor efficiency sets the PE start time.
    x_pack = nc.dram_tensor("x_pack", [QCH, 128, CT, 512], bf16, kind="ExternalInput")
    wq_pack = nc.dram_tensor("wq_pack", [HG, 128, CT, 128], bf16, kind="ExternalInput")
    wk_pack = nc.dram_tensor("wk_pack", [HG, 128, CT, 128], bf16, kind="ExternalInput")
    wv_pack = nc.dram_tensor("wv_pack", [VQ, 128, CT, 256], bf16, kind="ExternalInput")
    wo_pack = nc.dram_tensor("wo_pack", [128, HG, C], bf16, kind="ExternalInput")
    # cs_pack rows 0:64 = cos^T, rows 64:128 = sin^T
    cs_pack = nc.dram_tensor("cs_pack", [128, T], f32, kind="ExternalInput")
    out = nc.dram_tensor("out", [T, C], bf16, kind="ExternalOutput")

    with tile.TileContext(nc) as tc, ExitStack() as ctx:
        # Pools opened in lifetime order: persistent + attention scratch first
        # (bottom of the SBUF stack), then phase-1 pools on top, so attention
        # tiles never alias phase-1 space.
        persist = ctx.enter_context(tc.tile_pool(name="persist", bufs=1))
        p2k = ctx.enter_context(tc.tile_pool(name="p2k", bufs=2))
        p2q = ctx.enter_context(tc.tile_pool(name="p2q", bufs=2))
        p2u = ctx.enter_context(tc.tile_pool(name="p2u", bufs=2))
        p2sm = ctx.enter_context(tc.tile_pool(name="p2sm", bufs=1))
        p2v = ctx.enter_context(tc.tile_pool(name="p2v", bufs=1))
        dram = ctx.enter_context(tc.tile_pool(name="dram", bufs=1, space="DRAM"))
        ps_sc = ctx.enter_context(tc.tile_pool(name="ps_sc", bufs=2, space="PSUM"))
        ps_gen = ctx.enter_context(tc.tile_pool(name="ps_gen", bufs=3, space="PSUM"))
        ps_red = ctx.enter_context(tc.tile_pool(name="ps_red", bufs=1, space="PSUM"))

        yt_sb = persist.tile([128, HG, T], bf16)      # y^T, (d, h, t)
        ones_sb = persist.tile([128, 128], bf16, tag="ones")
        nc.vector.memset(ones_sb[:], 1.0)

        qt_dram = [
            dram.tile([128, T], f32r, tag=f"qt{h}", name=f"qt_dram{h}")
            for h in range(HG)
        ]
        v_dram = dram.tile([128, TT, JG], bf16, name="v_dram")

        # ---- attention micro-block emitters (head h, interleaved) --------
        class Att:
            def __init__(self, h, kt_sb):
                self.h = h
                self.kt = kt_sb
                v_h = p2v.tile([128, TT, 128], bf16, tag="vh", name=f"vh{h}")
                nc.sync.dma_start(out=v_h[:], in_=v_dram[:, :, ts(h, 128)])
                self.v = v_h
                self.qts = {}
                self.us = {}
                self.prefetch_qt(0)

            def prefetch_qt(self, qc):
                if qc >= QCH or qc in self.qts:
                    return
                qt = p2q.tile(
                    [128, 512], f32r, tag="qt", name=f"qt{self.h}_{qc}"
                )
                nc.sync.dma_start(
                    out=qt[:], in_=qt_dram[self.h][:, ts(qc, 512)]
                )
                self.qts[qc] = qt

            def sc(self, qc):
                h = self.h
                self.prefetch_qt(qc + 1)
                u = p2u.tile(
                    [128, KT // 2, 2, 512], bf16, tag="u", name=f"u{h}_{qc}"
                )
                self.us[qc] = u
                qt_r = self.qts[qc][:]
                for kg in range(KT // 2):
                    ps = ps_sc.tile(
                        [128, 2, 512], f32, tag="ps", name=f"sc{h}_{qc}_{kg}"
                    )
                    for kk in range(2):
                        nc.tensor.matmul(
                            ps[:, kk, :],
                            lhsT=self.kt[:, ts(2 * kg + kk, 128)],
                            rhs=qt_r,
                            start=True,
                            stop=True,
                        )
                    nc.scalar.activation(
                        out=u[:, kg, :, :],
                        in_=ps[:],
                        func=mybir.ActivationFunctionType.Exp,
                        scale=SCALE,
                    )

            def av(self, qc):
                h = self.h
                u = self.us.pop(qc)
                self.qts.pop(qc)
                # denominator tree-sum on VectorE; exp(qc) has drained by the
                # time this block is emitted, so these don't head-block the
                # (in-order) Vector queue
                s8 = p2sm.tile([128, 8, 512], bf16, tag="s8", name=f"s8_{h}{qc}")
                nc.vector.tensor_add(s8[:], u[:, :, 0, :], u[:, :, 1, :])
                s8v = s8[:].rearrange("p (x y) q -> p x y q", x=4)
                s4 = p2sm.tile([128, 4, 512], bf16, tag="s4", name=f"s4_{h}{qc}")
                nc.vector.tensor_add(s4[:], s8v[:, :, 0, :], s8v[:, :, 1, :])
                s4v = s4[:].rearrange("p (x y) q -> p x y q", x=2)
                s2r = p2sm.tile([128, 3, 512], bf16, tag="s2r", name=f"s2r_{h}{qc}")
                nc.vector.tensor_add(
                    s2r[:, 0:2, :], s4v[:, :, 0, :], s4v[:, :, 1, :]
                )
                nc.vector.tensor_add(s2r[:, 2, :], s2r[:, 0, :], s2r[:, 1, :])

                psy = ps_gen.tile([128, 512], f32, tag="ps", name=f"psy{h}_{qc}")
                for kt in range(KT):
                    nc.tensor.matmul(
                        psy[:],
                        lhsT=self.v[:, kt, :],
                        rhs=u[:, kt // 2, kt % 2, :],
                        start=(kt == 0),
                        stop=(kt == KT - 1),
                    )
                # cross-partition reduce of the denominator: one all-ones
                # matmul (~0.2us) instead of a ~3.5us gpsimd all-reduce
                rsum = ps_red.tile([128, 512], f32, tag="red", name=f"rs_{h}{qc}")
                nc.tensor.matmul(
                    rsum[:], lhsT=ones_sb[:], rhs=s2r[:, 2, :],
                    start=True, stop=True,
                )
                rrec = p2sm.tile([128, 512], f32, tag="s4", name=f"rr_{h}{qc}")
                nc.vector.reciprocal_approx_fast(out=rrec[:], in_=rsum[:])
                nc.vector.tensor_mul(
                    out=yt_sb[:, h, ts(qc, 512)], in0=psy[:], in1=rrec[:]
                )

        def oproj_group(g):
            # o-proj tile-group g needs q-chunk g of ALL heads; emitted in
            # the tail right after yt(7, g) is finalized.  Two 512-col
            # halves at a time so only 2 PSUM banks are held; each half's
            # output is DMA'd as soon as its copies finish.
            for tm in range(4 * g, 4 * g + 4):
                stg = p3stg.tile([128, C], bf16, tag="ostg", name=f"ostg{tm}")
                for half in range(2):
                    pss = [
                        ps_gen.tile(
                            [128, 512], f32, tag="ps", name=f"pso{tm}_{half}{c}"
                        )
                        for c in range(2)
                    ]
                    for ji in range(HG):
                        for c2 in range(2):
                            nc.tensor.matmul(
                                pss[c2][:],
                                lhsT=yt_sb[:, ji, ts(tm, 128)],
                                rhs=wo_sb[:, ji, ts(2 * half + c2, 512)],
                                start=(ji == 0),
                                stop=(ji == HG - 1),
                            )
                    for c2 in range(2):
                        cch = 2 * half + c2
                        nc.vector.tensor_copy(
                            out=stg[:, ts(cch, 512)], in_=pss[c2][:]
                        )
                        nc.sync.dma_start(
                            out=out.ap()[ts(tm, 128), ts(cch, 512)],
                            in_=stg[:, ts(cch, 512)],
                        )

        # ---- phase 1: projections with interleaved attention -------------
        with (
            tc.tile_pool(name="p1x", bufs=1) as p1x,
            tc.tile_pool(name="p1wv", bufs=2) as p1wv,
            tc.tile_pool(name="p1cs", bufs=1) as p1cs,
            tc.tile_pool(name="p1w", bufs=1) as p1w,
            tc.tile_pool(name="p1rot", bufs=1) as p1rot,
            tc.tile_pool(name="p1stg", bufs=2) as p1stg,
        ):
            def load_w(h):
                w_h = {}
                for nm, pack in (("q", wq_pack), ("k", wk_pack)):
                    w = p1w.tile(
                        [128, CT, 128], bf16, tag=f"w{nm}", name=f"w{nm}{h}"
                    )
                    nc.sync.dma_start(out=w[:], in_=pack.ap()[h])
                    w_h[nm] = w
                return w_h

            def load_wv(qd):
                wv_h = p1wv.tile(
                    [128, CT, 256], bf16, tag="wvh", name=f"wvh{qd}"
                )
                nc.sync.dma_start(out=wv_h[:], in_=wv_pack.ap()[qd])
                return wv_h

            # DMA issue order = DMA queue order (single in-order HW queue
            # striped over all 16 engines).  The first V chain needs only
            # wv quarter 0 + x chunk 0 (~2.6MB); later x chunks are emitted
            # lazily from inside v_quarter(0) so the V spill stores
            # interleave into the queue instead of stalling behind 8MB of
            # loads (vstg recycling would otherwise block the V chains).
            # first V chain needs only wv0[ci 0:8] + xc0[ci 0:8] (~1.3MB):
            # both are split in ci-halves so its first 8 matmuls start early
            # (subtile deps cover the mid-chain wait for the second half)
            wv_h0 = p1wv.tile([128, CT, 256], bf16, tag="wvh", name="wvh0")
            nc.sync.dma_start(out=wv_h0[:, 0:8, :], in_=wv_pack.ap()[0][:, 0:8, :])
            x_sb = p1x.tile([128, QCH, CT, 512], bf16, tag="xt")
            nc.sync.dma_start(out=x_sb[:, 0, 0:8, :], in_=x_pack.ap()[0][:, 0:8, :])
            nc.sync.dma_start(out=wv_h0[:, 8:16, :], in_=wv_pack.ap()[0][:, 8:16, :])
            nc.sync.dma_start(out=x_sb[:, 0, 8:16, :], in_=x_pack.ap()[0][:, 8:16, :])
            nc.sync.dma_start(out=x_sb[:, 1, :, :], in_=x_pack.ap()[1])
            wv_h1 = load_wv(1)
            # head-0 weights aren't needed until QK(0) (~75us in)
            w_next = load_w(0)

            def v_quarter(qd, wv_h, xload=None):
                # V columns [qd*256, qd*256+256) for all T (heads 2qd, 2qd+1)
                for tm in range(TT):
                    ps = ps_gen.tile(
                        [128, 256], f32, tag="ps", name=f"vps{qd}_{tm}"
                    )
                    for ci in range(CT):
                        nc.tensor.matmul(
                            ps[:],
                            lhsT=x_sb[:, tm // 4, ci, ts(tm % 4, 128)],
                            rhs=wv_h[:, ci, :],
                            start=(ci == 0),
                            stop=(ci == CT - 1),
                        )
                    vstg = p1stg.tile(
                        [128, 256], bf16, tag="vst", bufs=2,
                        name=f"vstg{qd}_{tm}"
                    )
                    nc.scalar.copy(out=vstg[:], in_=ps[:])
                    nc.sync.dma_start(
                        out=v_dram[:, tm, ts(qd, 256)], in_=vstg[:]
                    )
                    if xload and tm in xload:
                        tc4 = xload[tm]
                        nc.sync.dma_start(
                            out=x_sb[:, tc4, :, :], in_=x_pack.ap()[tc4]
                        )

            def qk_chain(h, w_h, kt_cur, nm, tch):
                ps = ps_gen.tile(
                    [128, 512], f32, tag="ps", name=f"qk{h}{nm}{tch}"
                )
                for ci in range(CT):
                    nc.tensor.matmul(
                        ps[:],
                        lhsT=w_h[nm][:, ci, :],
                        rhs=x_sb[:, tch, ci, :],
                        start=(ci == 0),
                        stop=(ci == CT - 1),
                    )
                # rotary: out1 = x1*cos + x2*sin ; out2 = x1*cos - x2*sin
                t12 = p1rot.tile([64, 2, 512], f32, tag="t12")
                t1 = t12[:, 0, :]
                t2 = t12[:, 1, :]
                nc.vector.tensor_mul(t1, ps[0:64, :], cs_sb[0:64, ts(tch, 512)])
                nc.vector.tensor_mul(
                    t2, ps[64:128, :], cs_sb[64:128, ts(tch, 512)]
                )
                if nm == "k":
                    # K^T written straight into its resident SBUF tile
                    nc.vector.tensor_add(kt_cur[0:64, ts(tch, 512)], t1, t2)
                    nc.vector.tensor_sub(kt_cur[64:128, ts(tch, 512)], t1, t2)
                else:
                    # Q^T spilled to DRAM in f32 (consumed as float32r)
                    stg = p1stg.tile(
                        [128, 512], f32r, tag="spl", bufs=1,
                        name=f"stg{h}{nm}{tch}"
                    )
                    nc.vector.tensor_add(stg[0:64, :], t1, t2)
                    nc.vector.tensor_sub(stg[64:128, :], t1, t2)
                    nc.sync.dma_start(
                        out=qt_dram[h][:, ts(tch, 512)], in_=stg[:]
                    )

            # V quarters 0,1 (heads 0-3) up front; 2,3 inside window 0
            v_quarter(0, wv_h0, xload={1: 2, 5: 3})
            # cos/sin only needed at the first rotary (~75us in)
            cs_sb = p1cs.tile([128, T], f32, tag="cs")
            nc.sync.dma_start(out=cs_sb[:], in_=cs_pack.ap())
            v_quarter(1, wv_h1)

            atts = {}
            kts = {}
            for h in range(HG):
                w_h = w_next
                kt_cur = p2k.tile([128, T], f32r, tag="kt", name=f"kt{h}")
                kts[h] = kt_cur
                # interleave schedule: after QK chain i of window h, emit
                # attention micro-block inserts[i].  AV(qc) trails SC(qc) by
                # >=3 chains (~15us) so the ScalarE exp has drained; the
                # last two AV blocks of head h-1 ride in window h+1.
                a_prev = atts.get(h - 2)   # AV(h-2, 2/3) pending
                a_cur = atts.get(h - 1)
                inserts = [
                    (a_prev, "av", 2),
                    (a_prev, "av", 3),
                    (a_cur, "sc", 0),
                    (a_cur, "sc", 1),
                    (a_cur, "av", 0),
                    (a_cur, "sc", 2),
                    (a_cur, "av", 1),
                    (a_cur, "sc", 3),
                ]
                chains = [(nm, tch) for nm in ("q", "k") for tch in range(QCH)]
                for i, (nm, tch) in enumerate(chains):
                    qk_chain(h, w_h, kt_cur, nm, tch)
                    a, kind, qc = inserts[i]
                    if a is not None:
                        getattr(a, kind)(qc)
                    if h == 0 and nm == "q" and tch == 3:
                        v_quarter(2, load_wv(2))
                    if h == 0 and nm == "k" and tch == 3:
                        v_quarter(3, load_wv(3))
                if h + 1 < HG:
                    w_next = load_w(h + 1)
                # issue attention(h)'s v_h/qt(0) loads now (kt/qt/v_dram for
                # head h are complete) — AFTER the w prefetch: the v_h DMA
                # blocks the in-order queue until AV(h-1,3) releases the
                # single v buffer, and w(h+1) must not sit behind that wait
                atts[h] = Att(h, kt_cur)
                if h - 2 in atts:
                    del atts[h - 2]

        # ---- tail: attention(7) + pending AV(6) + o-projection -----------
        p3wo = ctx.enter_context(tc.tile_pool(name="p3wo", bufs=1))
        p3stg = ctx.enter_context(tc.tile_pool(name="p3stg", bufs=2))

        a6 = atts[HG - 2]
        a7 = atts[HG - 1]
        # only qt(0)/qt(1) fit the 2-buf ring up front; a third prefetch's
        # DMA would wait on a buffer release at the HEAD of the in-order DMA
        # queue and block the wo loads behind it
        a7.prefetch_qt(1)
        wo_sb = p3wo.tile([128, HG, C], bf16)
        for ji in range(HG):
            nc.sync.dma_start(out=wo_sb[:, ji, :], in_=wo_pack.ap()[:, ji, :])

        a6.av(2)
        a7.sc(0)
        a6.av(3)
        a7.sc(1)
        a7.av(0)
        oproj_group(0)
        a7.sc(2)
        a7.av(1)
        oproj_group(1)
        a7.sc(3)
        a7.av(2)
        oproj_group(2)
        a7.av(3)
        oproj_group(3)

    nc.compile()
    return nc


def get_nc():
    if "nc" not in _CACHE:
        _CACHE["nc"] = _build_bass()
    return _CACHE["nc"]


def _pack_inputs(x, cos, sin, wq, wk, wv, wo):
    """Build the 8 per-core input maps (packed, DMA-friendly layouts)."""
    cs = np.concatenate(
        [
            np.asarray(cos[0, :, 0, :], dtype=np.float32).T,  # (64, T)
            np.asarray(sin[0, :, 0, :], dtype=np.float32).T,
        ],
        axis=0,
    )  # (128, T)
    cs = np.ascontiguousarray(cs)
    in_maps = []
    for core in range(N_CORES):
        b, g = divmod(core, 2)
        xb = np.asarray(x[b], dtype=np.float32)  # (T, C)
        # x_pack[tc4, p, ci, t'] = x[b, tc4*512+t', ci*128+p]
        x_pack = np.ascontiguousarray(
            xb.reshape(QCH, 512, CT, 128).transpose(0, 3, 2, 1).astype(BF16)
        )
        sl = slice(g * JG, (g + 1) * JG)
        wq_g = np.asarray(wq[sl], dtype=np.float32)  # (JG, C)
        wk_g = np.asarray(wk[sl], dtype=np.float32)
        wv_g = np.asarray(wv[sl], dtype=np.float32)
        wo_g = np.asarray(wo[:, sl], dtype=np.float32)  # (C, JG)
        # wq_pack[h, ci, co, d] = wq_g[h*128+d, co*128+ci]
        wq_pack = np.ascontiguousarray(
            wq_g.reshape(HG, 128, CT, 128).transpose(0, 3, 2, 1).astype(BF16)
        )
        wk_pack = np.ascontiguousarray(
            wk_g.reshape(HG, 128, CT, 128).transpose(0, 3, 2, 1).astype(BF16)
        )
        # wv_pack[qd, ci, co, d'] = wv_g[qd*256+d', co*128+ci]
        wv_pack = np.ascontiguousarray(
            wv_g.reshape(VQ, 256, CT, 128).transpose(0, 3, 2, 1).astype(BF16)
        )
        # wo_pack[ji, jo, c] = wo_g[c, jo*128+ji]
        wo_pack = np.ascontiguousarray(
            wo_g.reshape(C, HG, 128).transpose(2, 1, 0).astype(BF16)
        )
        in_maps.append(
            {
                "x_pack": x_pack,
                "wq_pack": wq_pack,
                "wk_pack": wk_pack,
                "wv_pack": wv_pack,
                "wo_pack": wo_pack,
                "cs_pack": cs,
            }
        )
    return in_maps


def run_spmd(in_maps, **kwargs):
    from concourse.bass_utils import run_bass_kernel_spmd

    nc = get_nc()
    return run_bass_kernel_spmd(nc, in_maps, core_ids=list(range(N_CORES)), **kwargs)


def kernel(x, cos, sin, wq, wk, wv, wo):
    in_maps = _pack_inputs(x, cos, sin, wq, wk, wv, wo)
    res = run_spmd(in_maps)
    outs = [np.asarray(r["out"], dtype=np.float32) for r in res.results]
    full = np.empty((B, T, C), dtype=np.float32)
    for b in range(B):
        full[b] = outs[2 * b] + outs[2 * b + 1]
    return full


# revision 15
# speedup vs baseline: 1.1873x; 1.0033x over previous
"""Bidirectional attention (RoPE-variant) Trainium2 kernel.

Reference computation (B=4, T=2048, C=2048, H=16, D=128):
    q = (x @ wq.T) -> rotary; k = (x @ wk.T) -> rotary; v = x @ wv.T
    y = softmax(q k^T / sqrt(D)) v ; out = y @ wo.T

Sharding over 8 NeuronCores: core c -> (batch b = c//2, head-group g = c%2).
Each core computes q/k/v projections for its batch restricted to its 8 heads,
full attention for those heads, and a partial o-projection (contracting its
1024 hidden columns).  The host sums the two partial outputs per batch — no
device collectives, and every core does exactly 1/8 of the matmul FLOPs.

Schedule: V is produced first (two wv quarters), then per head-window h the
Q/K projection chains for head h are emitted with attention micro-blocks for
head h-1 interleaved between them: SC(qc) = scores+exp for one 512-query
chunk, AV(qc) = tree-sum + attn@V + denominator-reduce + normalize.  The PE
queue is in-order, so this interleave is what lets the ScalarE exp() time
(~38us/window) hide under projection matmuls; AV(qc) is placed ~3 chains
after SC(qc) so exp has drained by then.  K^T stays resident in SBUF in f32
(rotary writes it directly; no spill), Q^T spills to DRAM in f32, and the
scores matmul consumes both as float32r — same PE throughput as bf16 at
N=512 (~227ns vs 216ns measured) with ~18x better precision, eliminating
the q/k quantization error that dominates exp(scores).  The softmax
denominator's cross-partition reduce is a single PE matmul against an
all-ones stationary operand.  The partial o-projection interleaves into the
last head's attention and is written out in bf16 (host accumulates in f32).
"""

import sys

if "/opt/trn_rl_repo" not in sys.path:
    sys.path.insert(0, "/opt/trn_rl_repo")

import numpy as np
import ml_dtypes

B, T, C = 4, 2048, 2048
H_TOT = 16
D = 128
HG = 8            # heads per core
JG = HG * D       # 1024 hidden columns per head-group
N_CORES = 8
CT = C // 128     # 16 c-tiles (contraction over channels)
TT = T // 128     # 16 t-tiles
QCH = T // 512    # 4 query chunks of 512
KT = T // 128     # 16 key tiles of 128
VQ = JG // 256    # 4 wv quarters
SCALE = 1.0 / float(np.sqrt(D))

BF16 = ml_dtypes.bfloat16

_CACHE = {}


def _build_bass():
    import concourse.tile as tile
    from concourse import bacc, mybir
    from concourse.bass import ts
    from contextlib import ExitStack

    bf16 = mybir.dt.bfloat16
    f32 = mybir.dt.float32
    f32r = mybir.dt.float32r

    nc = bacc.Bacc("TRN2", target_bir_lowering=False, debug=False)

    # x/wv are packed chunk-major so each load is one DMA with fat
    # per-partition-contiguous descriptors on both sides — startup is
    # DMA-bound, so descriptor efficiency sets the PE start time.
    x_pack = nc.dram_tensor("x_pack", [QCH, 128, CT, 512], bf16, kind="ExternalInput")
    wq_pack = nc.dram_tensor("wq_pack", [HG, 128, CT, 128], bf16, kind="ExternalInput")
    wk_pack = nc.dram_tensor("wk_pack", [HG, 128, CT, 128], bf16, kind="ExternalInput")
    wv_pack = nc.dram_tensor("wv_pack", [VQ, 128, CT, 256], bf16, kind="ExternalInput")
    wo_pack = nc.dram_tensor("wo_pack", [128, HG, C], bf16, kind="ExternalInput")
    # cs_pack rows 0:64 = cos^T, rows 64:128 = sin^T
    cs_pack = nc.dram_tensor("cs_pack", [128, T], f32, kind="ExternalInput")
    out = nc.dram_tensor("out", [T, C], bf16, kind="ExternalOutput")

    with tile.TileContext(nc) as tc, ExitStack() as ctx:
        # Pools opened in lifetime order: persistent + attention scratch first
        # (bottom of the SBUF stack), then phase-1 pools on top, so attention
        # tiles never alias phase-1 space.
        persist = ctx.enter_context(tc.tile_pool(name="persist", bufs=1))
        p2k = ctx.enter_context(tc.tile_pool(name="p2k", bufs=2))
        p2q = ctx.enter_context(tc.tile_pool(name="p2q", bufs=2))
        p2u = ctx.enter_context(tc.tile_pool(name="p2u", bufs=2))
        p2sm = ctx.enter_context(tc.tile_pool(name="p2sm", bufs=1))
        p2v = ctx.enter_context(tc.tile_pool(name="p2v", bufs=1))
        dram = ctx.enter_context(tc.tile_pool(name="dram", bufs=1, space="DRAM"))
        ps_sc = ctx.enter_context(tc.tile_pool(name="ps_sc", bufs=2, space="PSUM"))
        ps_gen = ctx.enter_context(tc.tile_pool(name="ps_gen", bufs=3, space="PSUM"))
        ps_red = ctx.enter_context(tc.tile_pool(name="ps_red", bufs=1, space="PSUM"))

        yt_sb = persist.tile([128, HG, T], bf16)      # y^T, (d, h, t)
        ones_sb = persist.tile([128, 128], bf16, tag="ones")
        nc.vector.memset(ones_sb[:], 1.0)

        qt_dram = [
            dram.tile([128, T], f32r, tag=f"qt{h}", name=f"qt_dram{h}")
            for h in range(HG)
        ]
        v_dram = dram.tile([128, TT, JG], bf16, name="v_dram")

        # ---- attention micro-block emitters (head h, interleaved) --------
        class Att:
            def __init__(self, h, kt_sb):
                self.h = h
                self.kt = kt_sb
                v_h = p2v.tile([128, TT, 128], bf16, tag="vh", name=f"vh{h}")
                nc.sync.dma_start(out=v_h[:], in_=v_dram[:, :, ts(h, 128)])
                self.v = v_h
                self.qts = {}
                self.us = {}
                self.prefetch_qt(0)

            def prefetch_qt(self, qc):
                if qc >= QCH or qc in self.qts:
                    return
                qt = p2q.tile(
                    [128, 512], f32r, tag="qt", name=f"qt{self.h}_{qc}"
                )
                nc.sync.dma_start(
                    out=qt[:], in_=qt_dram[self.h][:, ts(qc, 512)]
                )
                self.qts[qc] = qt

            def sc(self, qc):
                h = self.h
                self.prefetch_qt(qc + 1)
                u = p2u.tile(
                    [128, KT // 2, 2, 512], bf16, tag="u", name=f"u{h}_{qc}"
                )
                self.us[qc] = u
                qt_r = self.qts[qc][:]
                for kg in range(KT // 2):
                    ps = ps_sc.tile(
                        [128, 2, 512], f32, tag="ps", name=f"sc{h}_{qc}_{kg}"
                    )
                    for kk in range(2):
                        nc.tensor.matmul(
                            ps[:, kk, :],
                            lhsT=self.kt[:, ts(2 * kg + kk, 128)],
                            rhs=qt_r,
                            start=True,
                            stop=True,
                        )
                    nc.scalar.activation(
                        out=u[:, kg, :, :],
                        in_=ps[:],
                        func=mybir.ActivationFunctionType.Exp,
                        scale=SCALE,
                    )

            def av(self, qc):
                h = self.h
                u = self.us.pop(qc)
                self.qts.pop(qc)
                # denominator tree-sum on VectorE; exp(qc) has drained by the
                # time this block is emitted, so these don't head-block the
                # (in-order) Vector queue
                s8 = p2sm.tile([128, 8, 512], bf16, tag="s8", name=f"s8_{h}{qc}")
                nc.vector.tensor_add(s8[:], u[:, :, 0, :], u[:, :, 1, :])
                s8v = s8[:].rearrange("p (x y) q -> p x y q", x=4)
                s4 = p2sm.tile([128, 4, 512], bf16, tag="s4", name=f"s4_{h}{qc}")
                nc.vector.tensor_add(s4[:], s8v[:, :, 0, :], s8v[:, :, 1, :])
                s4v = s4[:].rearrange("p (x y) q -> p x y q", x=2)
                s2r = p2sm.tile([128, 3, 512], bf16, tag="s2r", name=f"s2r_{h}{qc}")
                nc.vector.tensor_add(
                    s2r[:, 0:2, :], s4v[:, :, 0, :], s4v[:, :, 1, :]
                )
                nc.vector.tensor_add(s2r[:, 2, :], s2r[:, 0, :], s2r[:, 1, :])

                psy = ps_gen.tile([128, 512], f32, tag="ps", name=f"psy{h}_{qc}")
                for kt in range(KT):
                    nc.tensor.matmul(
                        psy[:],
                        lhsT=self.v[:, kt, :],
                        rhs=u[:, kt // 2, kt % 2, :],
                        start=(kt == 0),
                        stop=(kt == KT - 1),
                    )
                # cross-partition reduce of the denominator: one all-ones
                # matmul (~0.2us) instead of a ~3.5us gpsimd all-reduce
                rsum = ps_red.tile([128, 512], f32, tag="red", name=f"rs_{h}{qc}")
                nc.tensor.matmul(
                    rsum[:], lhsT=ones_sb[:], rhs=s2r[:, 2, :],
                    start=True, stop=True,
                )
                rrec = p2sm.tile([128, 512], f32, tag="s4", name=f"rr_{h}{qc}")
                nc.vector.reciprocal_approx_fast(out=rrec[:], in_=rsum[:])
                nc.vector.tensor_mul(
                    out=yt_sb[:, h, ts(qc, 512)], in0=psy[:], in1=rrec[:]
                )

        def oproj_group(g):
            # o-proj tile-group g needs q-chunk g of ALL heads; emitted in
            # the tail right after yt(7, g) is finalized.  Two 512-col
            # halves at a time so only 2 PSUM banks are held; each half's
            # output is DMA'd as soon as its copies finish.
            for tm in range(4 * g, 4 * g + 4):
                stg = p3stg.tile([128, C], bf16, tag="ostg", name=f"ostg{tm}")
                for half in range(2):
                    pss = [
                        ps_gen.tile(
                            [128, 512], f32, tag="ps", name=f"pso{tm}_{half}{c}"
                        )
                        for c in range(2)
                    ]
                    for ji in range(HG):
                        for c2 in range(2):
                            nc.tensor.matmul(
                                pss[c2][:],
                                lhsT=yt_sb[:, ji, ts(tm, 128)],
                                rhs=wo_sb[:, ji, ts(2 * half + c2, 512)],
                                start=(ji == 0),
                                stop=(ji == HG - 1),
                            )
                    for c2 in range(2):
                        cch = 2 * half + c2
                        nc.vector.tensor_copy(
                            out=stg[:, ts(cch, 512)], in_=pss[c2][:]
                        )
                        nc.sync.dma_start(
                            out=out.ap()[ts(tm, 128), ts(cch, 512)],
                            in_=stg[:, ts(cch, 512)],
                        )

        # ---- phase 1: projections with interleaved attention -------------
        with (
            tc.tile_pool(name="p1x", bufs=1) as p1x,
            tc.tile_pool(name="p1wv", bufs=2) as p1wv,
            tc.tile_pool(name="p1cs", bufs=1) as p1cs,
            tc.tile_pool(name="p1w", bufs=1) as p1w,
            tc.tile_pool(name="p1rot", bufs=1) as p1rot,
            tc.tile_pool(name="p1stg", bufs=2) as p1stg,
        ):
            def load_w(h):
                w_h = {}
                for nm, pack in (("q", wq_pack), ("k", wk_pack)):
                    w = p1w.tile(
                        [128, CT, 128], bf16, tag=f"w{nm}", name=f"w{nm}{h}"
                    )
                    nc.sync.dma_start(out=w[:], in_=pack.ap()[h])
                    w_h[nm] = w
                return w_h

            def load_wv(qd):
                wv_h = p1wv.tile(
                    [128, CT, 256], bf16, tag="wvh", name=f"wvh{qd}"
                )
                nc.sync.dma_start(out=wv_h[:], in_=wv_pack.ap()[qd])
                return wv_h

            # DMA issue order = DMA queue order (single in-order HW queue
            # striped over all 16 engines).  The first V chain needs only
            # wv quarter 0 + x chunk 0 (~2.6MB); later x chunks are emitted
            # lazily from inside v_quarter(0) so the V spill stores
            # interleave into the queue instead of stalling behind 8MB of
            # loads (vstg recycling would otherwise block the V chains).
            # first V chain needs only wv0[ci 0:8] + xc0[ci 0:8] (~1.3MB):
            # both are split in ci-halves so its first 8 matmuls start early
            # (subtile deps cover the mid-chain wait for the second half)
            wv_h0 = p1wv.tile([128, CT, 256], bf16, tag="wvh", name="wvh0")
            nc.sync.dma_start(out=wv_h0[:, 0:8, :], in_=wv_pack.ap()[0][:, 0:8, :])
            x_sb = p1x.tile([128, QCH, CT, 512], bf16, tag="xt")
            nc.sync.dma_start(out=x_sb[:, 0, 0:8, :], in_=x_pack.ap()[0][:, 0:8, :])
            nc.sync.dma_start(out=wv_h0[:, 8:16, :], in_=wv_pack.ap()[0][:, 8:16, :])
            nc.sync.dma_start(out=x_sb[:, 0, 8:16, :], in_=x_pack.ap()[0][:, 8:16, :])
            nc.sync.dma_start(out=x_sb[:, 1, :, :], in_=x_pack.ap()[1])
            wv_h1 = load_wv(1)
            # head-0 weights aren't needed until QK(0) (~75us in)
            w_next = load_w(0)

            def v_quarter(qd, wv_h, xload=None):
                # V columns [qd*256, qd*256+256) for all T (heads 2qd, 2qd+1)
                for tm in range(TT):
                    ps = ps_gen.tile(
                        [128, 256], f32, tag="ps", name=f"vps{qd}_{tm}"
                    )
                    for ci in range(CT):
                        nc.tensor.matmul(
                            ps[:],
                            lhsT=x_sb[:, tm // 4, ci, ts(tm % 4, 128)],
                            rhs=wv_h[:, ci, :],
                            start=(ci == 0),
                            stop=(ci == CT - 1),
                        )
                    vstg = p1stg.tile(
                        [128, 256], bf16, tag="vst", bufs=2,
                        name=f"vstg{qd}_{tm}"
                    )
                    nc.scalar.copy(out=vstg[:], in_=ps[:])
                    nc.sync.dma_start(
                        out=v_dram[:, tm, ts(qd, 256)], in_=vstg[:]
                    )
                    if xload and tm in xload:
                        tc4 = xload[tm]
                        nc.sync.dma_start(
                            out=x_sb[:, tc4, :, :], in_=x_pack.ap()[tc4]
                        )

            def qk_chain(h, w_h, kt_cur, nm, tch):
                ps = ps_gen.tile(
                    [128, 512], f32, tag="ps", name=f"qk{h}{nm}{tch}"
                )
                for ci in range(CT):
                    nc.tensor.matmul(
                        ps[:],
                        lhsT=w_h[nm][:, ci, :],
                        rhs=x_sb[:, tch, ci, :],
                        start=(ci == 0),
                        stop=(ci == CT - 1),
                    )
                # rotary: out1 = x1*cos + x2*sin ; out2 = x1*cos - x2*sin
                t12 = p1rot.tile([64, 2, 512], f32, tag="t12")
                t1 = t12[:, 0, :]
                t2 = t12[:, 1, :]
                nc.vector.tensor_mul(t1, ps[0:64, :], cs_sb[0:64, ts(tch, 512)])
                nc.vector.tensor_mul(
                    t2, ps[64:128, :], cs_sb[64:128, ts(tch, 512)]
                )
                if nm == "k":
                    # K^T written straight into its resident SBUF tile
                    nc.vector.tensor_add(kt_cur[0:64, ts(tch, 512)], t1, t2)
                    nc.vector.tensor_sub(kt_cur[64:128, ts(tch, 512)], t1, t2)
                else:
                    # Q^T spilled to DRAM in f32 (consumed as float32r)
                    stg = p1stg.tile(
                        [128, 512], f32r, tag="spl", bufs=1,
                        name=f"stg{h}{nm}{tch}"
                    )
                    nc.vector.tensor_add(stg[0:64, :], t1, t2)
                    nc.vector.tensor_sub(stg[64:128, :], t1, t2)
                    nc.sync.dma_start(
                        out=qt_dram[h][:, ts(tch, 512)], in_=stg[:]
                    )

            # V quarters 0,1 (heads 0-3) up front; 2,3 inside window 0
            v_quarter(0, wv_h0, xload={1: 2, 5: 3})
            # cos/sin only needed at the first rotary (~75us in)
            cs_sb = p1cs.tile([128, T], f32, tag="cs")
            nc.sync.dma_start(out=cs_sb[:], in_=cs_pack.ap())
            v_quarter(1, wv_h1)

            atts = {}
            kts = {}
            for h in range(HG):
                w_h = w_next
                kt_cur = p2k.tile([128, T], f32r, tag="kt", name=f"kt{h}")
                kts[h] = kt_cur
                if h >= 1:
                    atts[h - 1] = Att(h - 1, kts[h - 1])
                # interleave schedule: after QK chain i of window h, emit
                # attention micro-block inserts[i].  AV(qc) trails SC(qc) by
                # >=3 chains (~15us) so the ScalarE exp has drained; the
                # last two AV blocks of head h-1 ride in window h+1.
                a_prev = atts.get(h - 2)   # AV(h-2, 2/3) pending
                a_cur = atts.get(h - 1)
                inserts = [
                    (a_prev, "av", 2),
                    (a_prev, "av", 3),
                    (a_cur, "sc", 0),
                    (a_cur, "sc", 1),
                    (a_cur, "av", 0),
                    (a_cur, "sc", 2),
                    (a_cur, "av", 1),
                    (a_cur, "sc", 3),
                ]
                chains = [(nm, tch) for nm in ("q", "k") for tch in range(QCH)]
                for i, (nm, tch) in enumerate(chains):
                    qk_chain(h, w_h, kt_cur, nm, tch)
                    a, kind, qc = inserts[i]
                    if a is not None:
                        getattr(a, kind)(qc)
                    if h == 0 and nm == "q" and tch == 3:
                        v_quarter(2, load_wv(2))
                    if h == 0 and nm == "k" and tch == 3:
                        v_quarter(3, load_wv(3))
                if h + 1 < HG:
                    w_next = load_w(h + 1)
                if h - 2 in atts:
                    del atts[h - 2]

        # ---- tail: attention(7) + pending AV(6) + o-projection -----------
        p3wo = ctx.enter_context(tc.tile_pool(name="p3wo", bufs=1))
        p3stg = ctx.enter_context(tc.tile_pool(name="p3stg", bufs=2))

        a6 = atts[HG - 2]
        a7 = Att(HG - 1, kts[HG - 1])
        # only qt(0)/qt(1) fit the 2-buf ring up front; a third prefetch's
        # DMA would wait on a buffer release at the HEAD of the in-order DMA
        # queue and block the wo loads behind it
        a7.prefetch_qt(1)
        wo_sb = p3wo.tile([128, HG, C], bf16)
        for ji in range(HG):
            nc.sync.dma_start(out=wo_sb[:, ji, :], in_=wo_pack.ap()[:, ji, :])

        a6.av(2)
        a7.sc(0)
        a6.av(3)
        a7.sc(1)
        a7.av(0)
        oproj_group(0)
        a7.sc(2)
        a7.av(1)
        oproj_group(1)
        a7.sc(3)
        a7.av(2)
        oproj_group(2)
        a7.av(3)
        oproj_group(3)

    nc.compile()
    return nc


def get_nc():
    if "nc" not in _CACHE:
        _CACHE["nc"] = _build_bass()
    return _CACHE["nc"]


def _pack_inputs(x, cos, sin, wq, wk, wv, wo):
    """Build the 8 per-core input maps (packed, DMA-friendly layouts)."""
    cs = np.concatenate(
        [
            np.asarray(cos[0, :, 0, :], dtype=np.float32).T,  # (64, T)
            np.asarray(sin[0, :, 0, :], dtype=np.float32).T,
        ],
        axis=0,
    )  # (128, T)
    cs = np.ascontiguousarray(cs)
    in_maps = []
    for core in range(N_CORES):
        b, g = divmod(core, 2)
        xb = np.asarray(x[b], dtype=np.float32)  # (T, C)
        # x_pack[tc4, p, ci, t'] = x[b, tc4*512+t', ci*128+p]
        x_pack = np.ascontiguousarray(
            xb.reshape(QCH, 512, CT, 128).transpose(0, 3, 2, 1).astype(BF16)
        )
        sl = slice(g * JG, (g + 1) * JG)
        wq_g = np.asarray(wq[sl], dtype=np.float32)  # (JG, C)
        wk_g = np.asarray(wk[sl], dtype=np.float32)
        wv_g = np.asarray(wv[sl], dtype=np.float32)
        wo_g = np.asarray(wo[:, sl], dtype=np.float32)  # (C, JG)
        # wq_pack[h, ci, co, d] = wq_g[h*128+d, co*128+ci]
        wq_pack = np.ascontiguousarray(
            wq_g.reshape(HG, 128, CT, 128).transpose(0, 3, 2, 1).astype(BF16)
        )
        wk_pack = np.ascontiguousarray(
            wk_g.reshape(HG, 128, CT, 128).transpose(0, 3, 2, 1).astype(BF16)
        )
        # wv_pack[qd, ci, co, d'] = wv_g[qd*256+d', co*128+ci]
        wv_pack = np.ascontiguousarray(
            wv_g.reshape(VQ, 256, CT, 128).transpose(0, 3, 2, 1).astype(BF16)
        )
        # wo_pack[ji, jo, c] = wo_g[c, jo*128+ji]
        wo_pack = np.ascontiguousarray(
            wo_g.reshape(C, HG, 128).transpose(2, 1, 0).astype(BF16)
        )
        in_maps.append(
            {
                "x_pack": x_pack,
                "wq_pack": wq_pack,
                "wk_pack": wk_pack,
                "wv_pack": wv_pack,
                "wo_pack": wo_pack,
                "cs_pack": cs,
            }
        )
    return in_maps


def run_spmd(in_maps, **kwargs):
    from concourse.bass_utils import run_bass_kernel_spmd

    nc = get_nc()
    return run_bass_kernel_spmd(nc, in_maps, core_ids=list(range(N_CORES)), **kwargs)


def kernel(x, cos, sin, wq, wk, wv, wo):
    in_maps = _pack_inputs(x, cos, sin, wq, wk, wv, wo)
    res = run_spmd(in_maps)
    outs = [np.asarray(r["out"], dtype=np.float32) for r in res.results]
    full = np.empty((B, T, C), dtype=np.float32)
    for b in range(B):
        full[b] = outs[2 * b] + outs[2 * b + 1]
    return full


# revision 16
# speedup vs baseline: 1.2012x; 1.0117x over previous
"""Bidirectional attention (RoPE-variant) Trainium2 kernel.

Reference computation (B=4, T=2048, C=2048, H=16, D=128):
    q = (x @ wq.T) -> rotary; k = (x @ wk.T) -> rotary; v = x @ wv.T
    y = softmax(q k^T / sqrt(D)) v ; out = y @ wo.T

Sharding over 8 NeuronCores: core c -> (batch b = c//2, head-group g = c%2).
Each core computes q/k/v projections for its batch restricted to its 8 heads,
full attention for those heads, and a partial o-projection (contracting its
1024 hidden columns).  The host sums the two partial outputs per batch — no
device collectives, and every core does exactly 1/8 of the matmul FLOPs.

Schedule: V is produced first (two wv quarters), then per head-window h the
Q/K projection chains for head h are emitted with attention micro-blocks for
head h-1 interleaved between them: SC(qc) = scores+exp for one 512-query
chunk, AV(qc) = tree-sum + attn@V + denominator-reduce + normalize.  The PE
queue is in-order, so this interleave is what lets the ScalarE exp() time
(~38us/window) hide under projection matmuls; AV(qc) is placed ~3 chains
after SC(qc) so exp has drained by then.  K^T stays resident in SBUF in f32
(rotary writes it directly; no spill), Q^T spills to DRAM in f32, and the
scores matmul consumes both as float32r — same PE throughput as bf16 at
N=512 (~227ns vs 216ns measured) with ~18x better precision, eliminating
the q/k quantization error that dominates exp(scores).  The softmax
denominator's cross-partition reduce is a single PE matmul against an
all-ones stationary operand.  The partial o-projection interleaves into the
last head's attention and is written out in bf16 (host accumulates in f32).
"""

import sys

if "/opt/trn_rl_repo" not in sys.path:
    sys.path.insert(0, "/opt/trn_rl_repo")

import numpy as np
import ml_dtypes

B, T, C = 4, 2048, 2048
H_TOT = 16
D = 128
HG = 8            # heads per core
JG = HG * D       # 1024 hidden columns per head-group
N_CORES = 8
CT = C // 128     # 16 c-tiles (contraction over channels)
TT = T // 128     # 16 t-tiles
QCH = T // 512    # 4 query chunks of 512
KT = T // 128     # 16 key tiles of 128
VQ = JG // 256    # 4 wv quarters
SCALE = 1.0 / float(np.sqrt(D))

BF16 = ml_dtypes.bfloat16

_CACHE = {}


def _build_bass():
    import concourse.tile as tile
    from concourse import bacc, mybir
    from concourse.bass import ts
    from contextlib import ExitStack

    bf16 = mybir.dt.bfloat16
    f32 = mybir.dt.float32
    f32r = mybir.dt.float32r

    nc = bacc.Bacc("TRN2", target_bir_lowering=False, debug=False)

    # x/wv are packed chunk-major so each load is one DMA with fat
    # per-partition-contiguous descriptors on both sides — startup is
    # DMA-bound, so descriptor efficiency sets the PE start time.
    x_pack = nc.dram_tensor("x_pack", [QCH, 128, CT, 512], bf16, kind="ExternalInput")
    wq_pack = nc.dram_tensor("wq_pack", [HG, 128, CT, 128], bf16, kind="ExternalInput")
    wk_pack = nc.dram_tensor("wk_pack", [HG, 128, CT, 128], bf16, kind="ExternalInput")
    wv_pack = nc.dram_tensor("wv_pack", [VQ, 128, CT, 256], bf16, kind="ExternalInput")
    wo_pack = nc.dram_tensor("wo_pack", [128, HG, C], bf16, kind="ExternalInput")
    # cs_pack rows 0:64 = cos^T, rows 64:128 = sin^T
    cs_pack = nc.dram_tensor("cs_pack", [128, T], f32, kind="ExternalInput")
    out = nc.dram_tensor("out", [T, C], bf16, kind="ExternalOutput")

    with tile.TileContext(nc) as tc, ExitStack() as ctx:
        # Pools opened in lifetime order: persistent + attention scratch first
        # (bottom of the SBUF stack), then phase-1 pools on top, so attention
        # tiles never alias phase-1 space.
        persist = ctx.enter_context(tc.tile_pool(name="persist", bufs=1))
        p2k = ctx.enter_context(tc.tile_pool(name="p2k", bufs=2))
        p2q = ctx.enter_context(tc.tile_pool(name="p2q", bufs=2))
        p2u = ctx.enter_context(tc.tile_pool(name="p2u", bufs=2))
        p2sm = ctx.enter_context(tc.tile_pool(name="p2sm", bufs=1))
        p2v = ctx.enter_context(tc.tile_pool(name="p2v", bufs=1))
        dram = ctx.enter_context(tc.tile_pool(name="dram", bufs=1, space="DRAM"))
        ps_sc = ctx.enter_context(tc.tile_pool(name="ps_sc", bufs=2, space="PSUM"))
        ps_gen = ctx.enter_context(tc.tile_pool(name="ps_gen", bufs=3, space="PSUM"))
        ps_red = ctx.enter_context(tc.tile_pool(name="ps_red", bufs=1, space="PSUM"))

        yt_sb = persist.tile([128, HG, T], bf16)      # y^T, (d, h, t)
        ones_sb = persist.tile([128, 128], bf16, tag="ones")
        nc.vector.memset(ones_sb[:], 1.0)

        qt_dram = [
            dram.tile([128, T], f32r, tag=f"qt{h}", name=f"qt_dram{h}")
            for h in range(HG)
        ]
        v_dram = dram.tile([128, TT, JG], bf16, name="v_dram")

        # ---- attention micro-block emitters (head h, interleaved) --------
        class Att:
            def __init__(self, h, kt_sb):
                self.h = h
                self.kt = kt_sb
                v_h = p2v.tile([128, TT, 128], bf16, tag="vh", name=f"vh{h}")
                nc.sync.dma_start(out=v_h[:], in_=v_dram[:, :, ts(h, 128)])
                self.v = v_h
                self.qts = {}
                self.us = {}
                self.prefetch_qt(0)

            def prefetch_qt(self, qc):
                if qc >= QCH or qc in self.qts:
                    return
                qt = p2q.tile(
                    [128, 512], f32r, tag="qt", name=f"qt{self.h}_{qc}"
                )
                nc.sync.dma_start(
                    out=qt[:], in_=qt_dram[self.h][:, ts(qc, 512)]
                )
                self.qts[qc] = qt

            def sc(self, qc):
                h = self.h
                self.prefetch_qt(qc + 1)
                u = p2u.tile(
                    [128, KT // 2, 2, 512], bf16, tag="u", name=f"u{h}_{qc}"
                )
                self.us[qc] = u
                qt_r = self.qts[qc][:]
                for kg in range(KT // 2):
                    ps = ps_sc.tile(
                        [128, 2, 512], f32, tag="ps", name=f"sc{h}_{qc}_{kg}"
                    )
                    for kk in range(2):
                        nc.tensor.matmul(
                            ps[:, kk, :],
                            lhsT=self.kt[:, ts(2 * kg + kk, 128)],
                            rhs=qt_r,
                            start=True,
                            stop=True,
                        )
                    nc.scalar.activation(
                        out=u[:, kg, :, :],
                        in_=ps[:],
                        func=mybir.ActivationFunctionType.Exp,
                        scale=SCALE,
                    )

            def av(self, qc):
                h = self.h
                u = self.us.pop(qc)
                self.qts.pop(qc)
                # denominator tree-sum on VectorE; exp(qc) has drained by the
                # time this block is emitted, so these don't head-block the
                # (in-order) Vector queue
                s8 = p2sm.tile([128, 8, 512], bf16, tag="s8", name=f"s8_{h}{qc}")
                nc.vector.tensor_add(s8[:], u[:, :, 0, :], u[:, :, 1, :])
                s8v = s8[:].rearrange("p (x y) q -> p x y q", x=4)
                s4 = p2sm.tile([128, 4, 512], bf16, tag="s4", name=f"s4_{h}{qc}")
                nc.vector.tensor_add(s4[:], s8v[:, :, 0, :], s8v[:, :, 1, :])
                s4v = s4[:].rearrange("p (x y) q -> p x y q", x=2)
                s2r = p2sm.tile([128, 3, 512], bf16, tag="s2r", name=f"s2r_{h}{qc}")
                nc.vector.tensor_add(
                    s2r[:, 0:2, :], s4v[:, :, 0, :], s4v[:, :, 1, :]
                )
                nc.vector.tensor_add(s2r[:, 2, :], s2r[:, 0, :], s2r[:, 1, :])

                psy = ps_gen.tile([128, 512], f32, tag="ps", name=f"psy{h}_{qc}")
                for kt in range(KT):
                    nc.tensor.matmul(
                        psy[:],
                        lhsT=self.v[:, kt, :],
                        rhs=u[:, kt // 2, kt % 2, :],
                        start=(kt == 0),
                        stop=(kt == KT - 1),
                    )
                # cross-partition reduce of the denominator: one all-ones
                # matmul (~0.2us) instead of a ~3.5us gpsimd all-reduce
                rsum = ps_red.tile([128, 512], f32, tag="red", name=f"rs_{h}{qc}")
                nc.tensor.matmul(
                    rsum[:], lhsT=ones_sb[:], rhs=s2r[:, 2, :],
                    start=True, stop=True,
                )
                rrec = p2sm.tile([128, 512], f32, tag="s4", name=f"rr_{h}{qc}")
                nc.vector.reciprocal_approx_fast(out=rrec[:], in_=rsum[:])
                nc.vector.tensor_mul(
                    out=yt_sb[:, h, ts(qc, 512)], in0=psy[:], in1=rrec[:]
                )

        def oproj_group(g):
            # o-proj tile-group g needs q-chunk g of ALL heads; emitted in
            # the tail right after yt(7, g) is finalized.  Two 512-col
            # halves at a time so only 2 PSUM banks are held; each half's
            # output is DMA'd as soon as its copies finish.
            for tm in range(4 * g, 4 * g + 4):
                stg = p3stg.tile([128, C], bf16, tag="ostg", name=f"ostg{tm}")
                for half in range(2):
                    pss = [
                        ps_gen.tile(
                            [128, 512], f32, tag="ps", name=f"pso{tm}_{half}{c}"
                        )
                        for c in range(2)
                    ]
                    for ji in range(HG):
                        for c2 in range(2):
                            nc.tensor.matmul(
                                pss[c2][:],
                                lhsT=yt_sb[:, ji, ts(tm, 128)],
                                rhs=wo_sb[:, ji, ts(2 * half + c2, 512)],
                                start=(ji == 0),
                                stop=(ji == HG - 1),
                            )
                    for c2 in range(2):
                        cch = 2 * half + c2
                        nc.vector.tensor_copy(
                            out=stg[:, ts(cch, 512)], in_=pss[c2][:]
                        )
                        nc.sync.dma_start(
                            out=out.ap()[ts(tm, 128), ts(cch, 512)],
                            in_=stg[:, ts(cch, 512)],
                        )

        # ---- phase 1: projections with interleaved attention -------------
        with (
            tc.tile_pool(name="p1x", bufs=1) as p1x,
            tc.tile_pool(name="p1wv", bufs=2) as p1wv,
            tc.tile_pool(name="p1cs", bufs=1) as p1cs,
            tc.tile_pool(name="p1w", bufs=1) as p1w,
            tc.tile_pool(name="p1rot", bufs=1) as p1rot,
            tc.tile_pool(name="p1stg", bufs=2) as p1stg,
        ):
            def load_w(h):
                w_h = {}
                for nm, pack in (("q", wq_pack), ("k", wk_pack)):
                    w = p1w.tile(
                        [128, CT, 128], bf16, tag=f"w{nm}", name=f"w{nm}{h}"
                    )
                    nc.sync.dma_start(out=w[:], in_=pack.ap()[h])
                    w_h[nm] = w
                return w_h

            def load_wv(qd):
                wv_h = p1wv.tile(
                    [128, CT, 256], bf16, tag="wvh", name=f"wvh{qd}"
                )
                nc.sync.dma_start(out=wv_h[:], in_=wv_pack.ap()[qd])
                return wv_h

            # DMA issue order = DMA queue order (single in-order HW queue
            # striped over all 16 engines).  The first V chain needs only
            # wv quarter 0 + x chunk 0 (~2.6MB); later x chunks are emitted
            # lazily from inside v_quarter(0) so the V spill stores
            # interleave into the queue instead of stalling behind 8MB of
            # loads (vstg recycling would otherwise block the V chains).
            wv_h0 = load_wv(0)
            x_sb = p1x.tile([128, QCH, CT, 512], bf16, tag="xt")
            nc.sync.dma_start(out=x_sb[:, 0, :, :], in_=x_pack.ap()[0])
            w_next = load_w(0)
            nc.sync.dma_start(out=x_sb[:, 1, :, :], in_=x_pack.ap()[1])
            wv_h1 = load_wv(1)

            def v_quarter(qd, wv_h, xload=None):
                # V columns [qd*256, qd*256+256) for all T (heads 2qd, 2qd+1)
                for tm in range(TT):
                    ps = ps_gen.tile(
                        [128, 256], f32, tag="ps", name=f"vps{qd}_{tm}"
                    )
                    for ci in range(CT):
                        nc.tensor.matmul(
                            ps[:],
                            lhsT=x_sb[:, tm // 4, ci, ts(tm % 4, 128)],
                            rhs=wv_h[:, ci, :],
                            start=(ci == 0),
                            stop=(ci == CT - 1),
                        )
                    vstg = p1stg.tile(
                        [128, 256], bf16, tag="vst", bufs=2,
                        name=f"vstg{qd}_{tm}"
                    )
                    nc.scalar.copy(out=vstg[:], in_=ps[:])
                    nc.sync.dma_start(
                        out=v_dram[:, tm, ts(qd, 256)], in_=vstg[:]
                    )
                    if xload and tm in xload:
                        tc4 = xload[tm]
                        nc.sync.dma_start(
                            out=x_sb[:, tc4, :, :], in_=x_pack.ap()[tc4]
                        )

            def qk_chain(h, w_h, kt_cur, nm, tch):
                ps = ps_gen.tile(
                    [128, 512], f32, tag="ps", name=f"qk{h}{nm}{tch}"
                )
                for ci in range(CT):
                    nc.tensor.matmul(
                        ps[:],
                        lhsT=w_h[nm][:, ci, :],
                        rhs=x_sb[:, tch, ci, :],
                        start=(ci == 0),
                        stop=(ci == CT - 1),
                    )
                # rotary: out1 = x1*cos + x2*sin ; out2 = x1*cos - x2*sin
                t12 = p1rot.tile([64, 2, 512], f32, tag="t12")
                t1 = t12[:, 0, :]
                t2 = t12[:, 1, :]
                nc.vector.tensor_mul(t1, ps[0:64, :], cs_sb[0:64, ts(tch, 512)])
                nc.vector.tensor_mul(
                    t2, ps[64:128, :], cs_sb[64:128, ts(tch, 512)]
                )
                if nm == "k":
                    # K^T written straight into its resident SBUF tile
                    nc.vector.tensor_add(kt_cur[0:64, ts(tch, 512)], t1, t2)
                    nc.vector.tensor_sub(kt_cur[64:128, ts(tch, 512)], t1, t2)
                else:
                    # Q^T spilled to DRAM in f32 (consumed as float32r)
                    stg = p1stg.tile(
                        [128, 512], f32r, tag="spl", bufs=1,
                        name=f"stg{h}{nm}{tch}"
                    )
                    nc.vector.tensor_add(stg[0:64, :], t1, t2)
                    nc.vector.tensor_sub(stg[64:128, :], t1, t2)
                    nc.sync.dma_start(
                        out=qt_dram[h][:, ts(tch, 512)], in_=stg[:]
                    )

            # V quarters 0,1 (heads 0-3) up front; 2,3 inside window 0
            v_quarter(0, wv_h0, xload={2: 2, 6: 3})
            # cos/sin only needed at the first rotary (~75us in)
            cs_sb = p1cs.tile([128, T], f32, tag="cs")
            nc.sync.dma_start(out=cs_sb[:], in_=cs_pack.ap())
            v_quarter(1, wv_h1)

            atts = {}
            kts = {}
            for h in range(HG):
                w_h = w_next
                kt_cur = p2k.tile([128, T], f32r, tag="kt", name=f"kt{h}")
                kts[h] = kt_cur
                if h >= 1:
                    atts[h - 1] = Att(h - 1, kts[h - 1])
                # interleave schedule: after QK chain i of window h, emit
                # attention micro-block inserts[i].  AV(qc) trails SC(qc) by
                # >=3 chains (~15us) so the ScalarE exp has drained; the
                # last two AV blocks of head h-1 ride in window h+1.
                a_prev = atts.get(h - 2)   # AV(h-2, 2/3) pending
                a_cur = atts.get(h - 1)
                inserts = [
                    (a_prev, "av", 2),
                    (a_prev, "av", 3),
                    (a_cur, "sc", 0),
                    (a_cur, "sc", 1),
                    (a_cur, "av", 0),
                    (a_cur, "sc", 2),
                    (a_cur, "av", 1),
                    (a_cur, "sc", 3),
                ]
                chains = [(nm, tch) for nm in ("q", "k") for tch in range(QCH)]
                for i, (nm, tch) in enumerate(chains):
                    qk_chain(h, w_h, kt_cur, nm, tch)
                    a, kind, qc = inserts[i]
                    if a is not None:
                        getattr(a, kind)(qc)
                    if h == 0 and nm == "q" and tch == 3:
                        v_quarter(2, load_wv(2))
                    if h == 0 and nm == "k" and tch == 3:
                        v_quarter(3, load_wv(3))
                if h + 1 < HG:
                    w_next = load_w(h + 1)
                if h - 2 in atts:
                    del atts[h - 2]

        # ---- tail: attention(7) + pending AV(6) + o-projection -----------
        p3wo = ctx.enter_context(tc.tile_pool(name="p3wo", bufs=1))
        p3stg = ctx.enter_context(tc.tile_pool(name="p3stg", bufs=2))

        a6 = atts[HG - 2]
        a7 = Att(HG - 1, kts[HG - 1])
        # only qt(0)/qt(1) fit the 2-buf ring up front; a third prefetch's
        # DMA would wait on a buffer release at the HEAD of the in-order DMA
        # queue and block the wo loads behind it
        a7.prefetch_qt(1)
        wo_sb = p3wo.tile([128, HG, C], bf16)
        for ji in range(HG):
            nc.sync.dma_start(out=wo_sb[:, ji, :], in_=wo_pack.ap()[:, ji, :])

        a6.av(2)
        a7.sc(0)
        a6.av(3)
        a7.sc(1)
        a7.av(0)
        oproj_group(0)
        a7.sc(2)
        a7.av(1)
        oproj_group(1)
        a7.sc(3)
        a7.av(2)
        oproj_group(2)
        a7.av(3)
        oproj_group(3)

    nc.compile()
    return nc


def get_nc():
    if "nc" not in _CACHE:
        _CACHE["nc"] = _build_bass()
    return _CACHE["nc"]


def _pack_inputs(x, cos, sin, wq, wk, wv, wo):
    """Build the 8 per-core input maps (packed, DMA-friendly layouts)."""
    cs = np.concatenate(
        [
            np.asarray(cos[0, :, 0, :], dtype=np.float32).T,  # (64, T)
            np.asarray(sin[0, :, 0, :], dtype=np.float32).T,
        ],
        axis=0,
    )  # (128, T)
    cs = np.ascontiguousarray(cs)
    in_maps = []
    for core in range(N_CORES):
        b, g = divmod(core, 2)
        xb = np.asarray(x[b], dtype=np.float32)  # (T, C)
        # x_pack[tc4, p, ci, t'] = x[b, tc4*512+t', ci*128+p]
        x_pack = np.ascontiguousarray(
            xb.reshape(QCH, 512, CT, 128).transpose(0, 3, 2, 1).astype(BF16)
        )
        sl = slice(g * JG, (g + 1) * JG)
        wq_g = np.asarray(wq[sl], dtype=np.float32)  # (JG, C)
        wk_g = np.asarray(wk[sl], dtype=np.float32)
        wv_g = np.asarray(wv[sl], dtype=np.float32)
        wo_g = np.asarray(wo[:, sl], dtype=np.float32)  # (C, JG)
        # wq_pack[h, ci, co, d] = wq_g[h*128+d, co*128+ci]
        wq_pack = np.ascontiguousarray(
            wq_g.reshape(HG, 128, CT, 128).transpose(0, 3, 2, 1).astype(BF16)
        )
        wk_pack = np.ascontiguousarray(
            wk_g.reshape(HG, 128, CT, 128).transpose(0, 3, 2, 1).astype(BF16)
        )
        # wv_pack[qd, ci, co, d'] = wv_g[qd*256+d', co*128+ci]
        wv_pack = np.ascontiguousarray(
            wv_g.reshape(VQ, 256, CT, 128).transpose(0, 3, 2, 1).astype(BF16)
        )
        # wo_pack[ji, jo, c] = wo_g[c, jo*128+ji]
        wo_pack = np.ascontiguousarray(
            wo_g.reshape(C, HG, 128).transpose(2, 1, 0).astype(BF16)
        )
        in_maps.append(
            {
                "x_pack": x_pack,
                "wq_pack": wq_pack,
                "wk_pack": wk_pack,
                "wv_pack": wv_pack,
                "wo_pack": wo_pack,
                "cs_pack": cs,
            }
        )
    return in_maps


def run_spmd(in_maps, **kwargs):
    from concourse.bass_utils import run_bass_kernel_spmd

    nc = get_nc()
    return run_bass_kernel_spmd(nc, in_maps, core_ids=list(range(N_CORES)), **kwargs)


def kernel(x, cos, sin, wq, wk, wv, wo):
    in_maps = _pack_inputs(x, cos, sin, wq, wk, wv, wo)
    res = run_spmd(in_maps)
    outs = [np.asarray(r["out"], dtype=np.float32) for r in res.results]
    full = np.empty((B, T, C), dtype=np.float32)
    for b in range(B):
        full[b] = outs[2 * b] + outs[2 * b + 1]
    return full
